# revision 10
# baseline (speedup 1.0000x reference)
"""CrossNet kernel for Trainium2, data-parallel over 8 NeuronCores.

Reference computation (per layer l = 0..3):
    s_l  = xl . W[l]                (per-row scalar)
    xl  <- x0 * s_l + b[l] + xl

Key algebraic collapse: xl always stays in the affine form
    xl_l = x0 * alpha_l + beta_l
with alpha_l a per-row scalar and beta_l a per-layer constant vector:
    alpha_0 = 1,  beta_0 = 0
    s_l       = alpha_l * p_l + q_l,   p_l = x0 . W[l]  (per-row),
                                       q_l = beta_l . W[l]  (host scalar)
    alpha_{l+1} = alpha_l * (1 + p_l) + q_l
    beta_{l+1}  = beta_l + b[l]
so the whole network needs just one skinny matmul P = x0 @ W^T, a
4-step per-row recurrence, and one fused output pass
    out = x0 * alpha_4 + beta_4.

Device mapping per 128-row tile:
    PE   : 8x transpose(128x128) -> XT, ones-matmul + 8x matmul (W^T chunk
           stationary, XT chunk moving) accumulating (1+p)^T[4,128] in PSUM,
           1 small back-transpose to [128,4]
    ACT  : PSUM->SBUF copies (XT, PT), output DMA issue (ACT HWDGE queue)
    DVE  : alpha recurrence (tensor_tensor_scan), fused
           out = (x0 * alpha) + beta4 (scalar_tensor_tensor)
    DMA  : input stream on SP HWDGE queue (all loads queued up front),
           output stream on ACT HWDGE queue; ~16MB/core = the roofline
"""

import numpy as np

import concourse.bacc as bacc
import concourse.bass as bass
import concourse.tile as tile
from concourse import mybir
from concourse.bass_utils import run_bass_kernel_spmd

BATCH = 16384
DIM = 1024
NUM_LAYERS = 4
NCORES = 8
SHARD = BATCH // NCORES  # 2048
P = 128
NT = SHARD // P          # 16 row-tiles per core
SUB = 4                  # row-tiles per group (batched N=512 matmuls)
NST = NT // SUB          # 4 groups
NB = SUB * P             # 512 batched rows per group
NCHUNK = DIM // P        # 8 contraction chunks

_F32 = mybir.dt.float32

_cached_nc = None


def _build_program():
    nc = bacc.Bacc(None)

    x = nc.declare_dram_parameter("x", [SHARD, DIM], _F32, isOutput=False)
    wt = nc.declare_dram_parameter("wt", [P, NCHUNK * NUM_LAYERS], _F32, isOutput=False)
    qrow = nc.declare_dram_parameter("qrow", [1, NUM_LAYERS], _F32, isOutput=False)
    beta4 = nc.declare_dram_parameter("beta4", [1, DIM], _F32, isOutput=False)
    id128 = nc.declare_dram_parameter("id128", [P, P], _F32, isOutput=False)
    id4 = nc.declare_dram_parameter("id4", [NUM_LAYERS, NUM_LAYERS], _F32, isOutput=False)
    out = nc.declare_dram_parameter("out", [SHARD, DIM], _F32, isOutput=True)

    x_t = x.rearrange("(n s p) d -> n s p d", s=SUB, p=P)
    out_t = out.rearrange("(n s p) d -> n s p d", s=SUB, p=P)

    def bcast(ap, n):
        # read a [1, F] DRAM row broadcast onto n partitions
        return bass.AP(tensor=ap.tensor, offset=ap.offset, ap=[[0, n]] + list(ap.ap[1:]))

    with (
        tile.TileContext(nc) as tc,
        tc.tile_pool(name="consts", bufs=1) as consts,
        tc.tile_pool(name="xs", bufs=NST) as xs,
        tc.tile_pool(name="xts", bufs=2) as xts,
        tc.tile_pool(name="outs", bufs=2) as outs,
        tc.tile_pool(name="small", bufs=4) as small,
        tc.tile_pool(name="ps_xt", bufs=2, space="PSUM") as ps_xt,
        tc.tile_pool(name="ps_pt", bufs=2, space="PSUM") as ps_pt,
        tc.tile_pool(name="ps_p", bufs=2, space="PSUM") as ps_p,
    ):
        wt_sb = consts.tile([P, NCHUNK * NUM_LAYERS], _F32)
        nc.sync.dma_start(out=wt_sb, in_=wt[:])
        qrow_sb = consts.tile([P, NUM_LAYERS], _F32)
        nc.sync.dma_start(out=qrow_sb, in_=bcast(qrow[:], P))
        beta4_sb = consts.tile([P, DIM], _F32)
        nc.sync.dma_start(out=beta4_sb, in_=bcast(beta4[:], P))
        id128_sb = consts.tile([P, P], _F32)
        nc.sync.dma_start(out=id128_sb, in_=id128[:])
        id4_sb = consts.tile([NUM_LAYERS, NUM_LAYERS], _F32)
        nc.sync.dma_start(out=id4_sb, in_=id4[:])
        ones14_sb = consts.tile([1, NUM_LAYERS], _F32)
        nc.vector.memset(ones14_sb, 1.0)
        ones1n_sb = consts.tile([1, NB], _F32)
        nc.vector.memset(ones1n_sb, 1.0)

        # queue every input load up front on the SP HWDGE queue
        X_tiles = []
        for st in range(NST):
            X = xs.tile([P, SUB, DIM], _F32)
            nc.sync.dma_start(out=X, in_=x_t[st])
            X_tiles.append(X)

        for st in range(NST):
            X = X_tiles[st]
            O = outs.tile([P, SUB, DIM], _F32)
            # XT4[d_in_chunk, c, s*128+b] = X[b, s, c*128+d] for the group's
            # 512 rows, so each contraction chunk is one N=512 matmul.
            XT4 = xts.tile([P, NCHUNK, NB], _F32)
            for sub in range(SUB):
                Xs = X[:, sub, :]
                XT_ps = ps_xt.tile([P, DIM], _F32)
                for c in range(NCHUNK):
                    nc.tensor.transpose(
                        XT_ps[:, c * P:(c + 1) * P], Xs[:, c * P:(c + 1) * P], id128_sb
                    )
                nc.scalar.copy(
                    XT4[:, :, sub * P:(sub + 1) * P],
                    XT_ps.rearrange("p (c b) -> p c b", c=NCHUNK),
                )

            # PT4[l, n] = 1 + sum_d W[l, d] * XT4[d, :, n], N=512 per matmul
            # (the leading ones-matmul seeds the +1 for the recurrence)
            PT_ps = ps_pt.tile([NUM_LAYERS, NB], _F32)
            nc.tensor.matmul(PT_ps, ones14_sb, ones1n_sb, start=True, stop=False)
            for c in range(NCHUNK):
                nc.tensor.matmul(
                    PT_ps,
                    wt_sb[:, c * NUM_LAYERS:(c + 1) * NUM_LAYERS],
                    XT4[:, c, :],
                    start=False,
                    stop=(c == NCHUNK - 1),
                )
            PT = small.tile([NUM_LAYERS, NB], _F32)
            nc.scalar.copy(PT, PT_ps)

            for sub in range(SUB):
                # back to [b, l] layout for the per-row recurrence
                P_ps = ps_p.tile([P, NUM_LAYERS], _F32)
                nc.tensor.transpose(
                    P_ps, PT[:, sub * P:(sub + 1) * P], id4_sb
                )

                # alpha_{l+1} = alpha_l * (1 + p_l) + q_l, alpha_0 = 1
                AL = small.tile([P, NUM_LAYERS], _F32)
                nc.vector.tensor_tensor_scan(
                    AL, P_ps, qrow_sb, 1.0, mybir.AluOpType.mult, mybir.AluOpType.add
                )

                # out = x0 * alpha_4 + beta_4, fused in one DVE op
                nc.vector.scalar_tensor_tensor(
                    O[:, sub, :], X[:, sub, :], AL[:, NUM_LAYERS - 1:NUM_LAYERS],
                    beta4_sb, mybir.AluOpType.mult, mybir.AluOpType.add,
                )
            # output DMA on the ACT HWDGE queue (separate from input stream)
            nc.scalar.dma_start(out=out_t[st], in_=O)

    nc.compile()
    return nc


def _host_constants(W, b):
    W64 = W.astype(np.float64)
    b64 = b.astype(np.float64)
    q = np.zeros(NUM_LAYERS, dtype=np.float64)
    beta = np.zeros(DIM, dtype=np.float64)
    for l in range(NUM_LAYERS):
        q[l] = beta @ W64[l]
        beta += b64[l]
    # wt[k, c*4 + l] = W[l, c*128 + k]
    wt = np.ascontiguousarray(
        W.T.reshape(NCHUNK, P, NUM_LAYERS).transpose(1, 0, 2).reshape(P, NCHUNK * NUM_LAYERS)
    ).astype(np.float32)
    qrow = q.astype(np.float32).reshape(1, NUM_LAYERS)
    beta4 = beta.astype(np.float32).reshape(1, DIM)
    id128 = np.eye(P, dtype=np.float32)
    id4 = np.eye(NUM_LAYERS, dtype=np.float32)
    return wt, qrow, beta4, id128, id4


def _run(x0, W, b, trace=False):
    global _cached_nc
    if _cached_nc is None:
        _cached_nc = _build_program()
    nc = _cached_nc

    x0 = np.ascontiguousarray(x0, dtype=np.float32)
    wt, qrow, beta4, id128, id4 = _host_constants(
        np.asarray(W, dtype=np.float32), np.asarray(b, dtype=np.float32)
    )
    shards = x0.reshape(NCORES, SHARD, DIM)
    in_maps = [
        {"x": shards[i], "wt": wt, "qrow": qrow, "beta4": beta4,
         "id128": id128, "id4": id4}
        for i in range(NCORES)
    ]
    res = run_bass_kernel_spmd(nc, in_maps, list(range(NCORES)), trace=trace)
    out = np.concatenate([res.results[i]["out"] for i in range(NCORES)], axis=0)
    return out, res


def kernel(x0, W, b):
    out, _ = _run(x0, W, b, trace=False)
    return out


def _register_ntff_hook():
    """The container's antenv stub lacks axon_hooks; replicate the boot-time
    ctypes NTFF hook (see trn_boot._ntff_profile_via_ctypes) so trace=True
    can capture HW profiles."""
    import sys
    import types
    import ctypes
    import contextlib

    if "antenv.axon_hooks" in sys.modules:
        return
    so_path = "/opt/axon/libaxon_pjrt.so"
    lib = ctypes.CDLL(so_path)
    if not hasattr(lib, "axon_start_nrt_profile"):
        return
    lib.axon_start_nrt_profile.argtypes = [
        ctypes.POINTER(ctypes.c_int64),
        ctypes.c_size_t,
    ]
    lib.axon_start_nrt_profile.restype = ctypes.c_int64
    lib.axon_stop_nrt_profile.argtypes = [ctypes.c_char_p]
    lib.axon_stop_nrt_profile.restype = ctypes.c_int64

    @contextlib.contextmanager
    def _hook(output_dir, device_ids):
        import jax

        jax.devices()
        if device_ids:
            ids = (ctypes.c_int64 * len(device_ids))(*device_ids)
            rc = lib.axon_start_nrt_profile(ids, len(device_ids))
        else:
            rc = lib.axon_start_nrt_profile(None, 0)
        if rc != 0:
            raise RuntimeError(f"axon_start_nrt_profile rc={rc}")
        try:
            yield
        finally:
            n = lib.axon_stop_nrt_profile(str(output_dir).encode())
            print(f"ntff profile: {n} file(s) written to {output_dir}")

    mod = types.ModuleType("antenv.axon_hooks")
    mod.get_axon_ntff_profile_hook = lambda: _hook
    mod.set_axon_ntff_profile_hook = lambda h: None
    sys.modules["antenv.axon_hooks"] = mod


def kernel_timed(x0, W, b):
    _register_ntff_hook()
    out, res = _run(x0, W, b, trace=True)
    return out, res
